# revision 8
# baseline (speedup 1.0000x reference)
"""Trainium2 Bass kernel for the Hoyer-spike attention module (B=8,N=1024,C=768,H=12).

Math (per batch, per head): xf = spike1(x); [q|k|v] = xf @ qkv_w.T; ks,vs =
spike2(k),spike2(v) (binary); y = q @ (ks.T @ vs) (exact reassociation of
(q@ks.T)@vs -- no softmax); z = spike3(y) with torch's reshape(B,H,D,N)
reinterpretation; out = z @ proj_w.T + proj_b.

Distribution: data-parallel over B=8 -> one batch per NeuronCore, weights
replicated, no collectives.

Numerics: qkv weights split hi+lo bf16 and accumulated in one PSUM group
(xf binary => products exact; ~17-bit effective weights). M = ks.T@vs is
exact small integers. y-matmul in plain fp32. proj weights single bf16.
BN+Hoyer affine transforms are folded host-side: into the k/v weight rows
(scale) + per-column thresholds, into the x/q copies (per-partition scale)
+ per-partition thresholds -- every spike is a single DVE op.

Layouts: x and weights host-transposed; xfT then serves both as stationary
operand (natural-layout k|v) and moving operand (transposed qT). The torch
reshape shuffle is absorbed into the qT PSUM->SBUF copy via a rearranged
output access pattern, so the y-matmul uses contiguous operands and its
spike lands directly in zT layout for the proj matmul. Odd heads are
re-based to partition 0 with small SBUF->SBUF DMAs (DMA moves across
partitions; DVE cannot).
"""
import sys
sys.path.insert(0, '/opt/trn_rl_repo')
import numpy as np
import ml_dtypes

import concourse.bass as bass
import concourse.mybir as mybir
import concourse.tile as tile
from concourse import bacc

F32 = mybir.dt.float32
BF16 = mybir.dt.bfloat16
FP16 = mybir.dt.float16
AOT = mybir.AluOpType

B, N, C, H, D = 8, 1024, 768, 12, 64
EPS, XS = 1e-5, 1.0
NCORES = 8
BF = np.dtype(ml_dtypes.bfloat16)


def build_nc():
    nc = bacc.Bacc(None, target_bir_lowering=False)
    xt_d = nc.declare_dram_parameter("xt", [C, N], F32, isOutput=False)
    whi_d = nc.declare_dram_parameter("w_hi", [C, 3 * C], BF16, isOutput=False)
    wlo_d = nc.declare_dram_parameter("w_lo", [C, 3 * C], BF16, isOutput=False)
    phi_d = nc.declare_dram_parameter("p_hi", [C, C], BF16, isOutput=False)
    txa_d = nc.declare_dram_parameter("txA", [128, 6], F32, isOutput=False)
    txt_d = nc.declare_dram_parameter("txT", [128, 6], F32, isOutput=False)
    tkv_d = nc.declare_dram_parameter("tkv", [128, 2 * C], F32, isOutput=False)
    qsc_d = nc.declare_dram_parameter("qsc", [128, 6], F32, isOutput=False)
    tyo_d = nc.declare_dram_parameter("tyO", [128, H], F32, isOutput=False)
    pb_d = nc.declare_dram_parameter("pb", [128, C], F32, isOutput=False)
    out_d = nc.declare_dram_parameter("out", [N, C], F32, isOutput=True)

    with tile.TileContext(nc) as tc:
        with (
            tc.tile_pool(name="const", bufs=1) as const,
            tc.tile_pool(name="work", bufs=3) as work,
            tc.tile_pool(name="mm", bufs=4, space="PSUM") as mm,
            tc.tile_pool(name="mps", bufs=1, space="PSUM") as mps,
        ):
            # ---- constants ----
            txa = const.tile([128, 6], F32, name="txa")
            txt = const.tile([128, 6], F32, name="txt")
            tkv = const.tile([128, 2 * C], F32, name="tkv")
            qsc = const.tile([128, 6], F32, name="qsc")
            tyo = const.tile([128, H], F32, name="tyo")
            pb = const.tile([128, C], F32, name="pb")
            nc.sync.dma_start(txa[:], txa_d[:])
            nc.sync.dma_start(txt[:], txt_d[:])

            w_hi = [const.tile([128, 3 * C], BF16, name=f"whi{ck}") for ck in range(6)]
            w_lo = [const.tile([128, 3 * C], BF16, name=f"wlo{ck}") for ck in range(6)]
            p_hi = [const.tile([128, C], BF16, name=f"phi{ck}") for ck in range(6)]

            # ---- phase 1: xT -> spike -> xfT (bf16 binary) ----
            # DMA order follows first use: x chunks + q-columns of w_hi first
            # (phase 2 can start), then w_lo q-cols, then k|v columns, proj
            # weights last.
            xf = [const.tile([128, N], BF16, name=f"xf{ck}") for ck in range(6)]
            for ck in range(6):
                xts = work.tile([128, N], F32, name=f"xts{ck}", tag="xt")
                nc.sync.dma_start(xts[:], xt_d[ck * 128:(ck + 1) * 128, :])
                nc.sync.dma_start(w_hi[ck][:, 0:C], whi_d[ck * 128:(ck + 1) * 128, 0:C])
                nc.vector.tensor_scalar(xf[ck][:], xts[:],
                                        txa[:, ck:ck + 1], txt[:, ck:ck + 1],
                                        AOT.mult, AOT.is_ge)
            nc.sync.dma_start(qsc[:], qsc_d[:])
            for ck in range(6):
                nc.sync.dma_start(w_lo[ck][:, 0:C], wlo_d[ck * 128:(ck + 1) * 128, 0:C])
            for ck in range(6):
                nc.sync.dma_start(w_hi[ck][:, C:3 * C],
                                  whi_d[ck * 128:(ck + 1) * 128, C:3 * C])
            for ck in range(6):
                nc.sync.dma_start(w_lo[ck][:, C:3 * C],
                                  wlo_d[ck * 128:(ck + 1) * 128, C:3 * C])
            nc.sync.dma_start(tkv[:], tkv_d[:])
            nc.sync.dma_start(tyo[:], tyo_d[:])
            for ck in range(6):
                nc.sync.dma_start(p_hi[ck][:], phi_d[ck * 128:(ck + 1) * 128, :])
            nc.sync.dma_start(pb[:], pb_d[:])

            # ---- phase 2: qT (shuffled layout) = A_o * (Wq @ xfT) ----
            # chunk hp holds heads (2hp, 2hp+1) on partitions 0:64 / 64:128.
            # Shuffled free axis: col m = (n%16)*64 + n//16 so the y-matmul
            # lhsT slices are contiguous.
            qT = [const.tile([128, N], FP16, name=f"qT{hp}") for hp in range(6)]
            qTo = [const.tile([64, N], FP16, name=f"qTo{hp}") for hp in range(6)]
            for hp in range(6):
                for nf in range(2):
                    p = mm.tile([128, 512], F32, name=f"qp{hp}_{nf}", tag="mm")
                    for hl, wgt in enumerate((w_hi, w_lo)):
                        for ck in range(6):
                            nc.tensor.matmul(p[:], wgt[ck][:, hp * 128:(hp + 1) * 128],
                                             xf[ck][:, nf * 512:(nf + 1) * 512],
                                             start=(hl == 0 and ck == 0),
                                             stop=(hl == 1 and ck == 5))
                    src = p[:, :].rearrange("p (a b) -> p a b", a=32)
                    dst = qT[hp][:, :].rearrange("p (b a) -> p a b", b=16)[:, nf * 32:(nf + 1) * 32, :]
                    nc.vector.tensor_scalar(dst, src, qsc[:, hp:hp + 1], None, AOT.mult)
                # odd head re-based to partitions 0:64 (DMA shifts partitions)
                nc.sync.dma_start(qTo[hp][:, :], qT[hp][64:128, :])

            # ---- phase 3: k|v chunks + spikes + M accumulation ----
            m_ps = mps.tile([64, H * D], F32, name="m_ps")   # all heads along free
            for nk in range(8):
                kvs = work.tile([128, 2 * C], BF16, name=f"kvs{nk}", tag="kvs")
                for kvf in range(3):
                    p = mm.tile([128, 512], F32, name=f"kvp{nk}_{kvf}", tag="mm")
                    for hl, wgt in enumerate((w_hi, w_lo)):
                        for ck in range(6):
                            nc.tensor.matmul(p[:], xf[ck][:, nk * 128:(nk + 1) * 128],
                                             wgt[ck][:, C + kvf * 512: C + (kvf + 1) * 512],
                                             start=(hl == 0 and ck == 0),
                                             stop=(hl == 1 and ck == 5))
                    nc.vector.tensor_tensor(kvs[:, kvf * 512:(kvf + 1) * 512], p[:],
                                            tkv[:, kvf * 512:(kvf + 1) * 512], AOT.is_ge)
                # PSUM accumulation groups are per-bank: m_ps spans 2 banks
                # (heads 0-7 / 8-11); open each bank's group on its first
                # matmul, close on its last.
                for h in range(H):
                    nc.tensor.matmul(m_ps[:, h * 64:(h + 1) * 64],
                                     kvs[:, h * 64:(h + 1) * 64],
                                     kvs[:, C + h * 64: C + (h + 1) * 64],
                                     start=(nk == 0 and h in (0, 8)),
                                     stop=(nk == 7 and h in (7, 11)))

            # ---- phase 4: y-matmul -> spike -> zT ----
            m_lo = const.tile([64, H * D], FP16, name="m_lo")
            nc.vector.tensor_copy(m_lo[:], m_ps[:])
            z_bf = [const.tile([128, N], BF16, name=f"z{hp}") for hp in range(6)]
            for hp in range(6):
                for pr in range(2):
                    h = 2 * hp + pr
                    lhs = qT[hp] if pr == 0 else qTo[hp]
                    ztmp = (z_bf[hp] if pr == 0 else
                            work.tile([64, N], BF16, name=f"zt{h}", tag="ztmp"))
                    for half in range(2):
                        zp = mm.tile([64, 512], F32, name=f"zp{h}_{half}", tag="mm")
                        for q8 in range(8):
                            qb = half * 8 + q8
                            nc.tensor.matmul(zp[:, q8 * 64:(q8 + 1) * 64],
                                             lhs[0:64, qb * 64:(qb + 1) * 64],
                                             m_lo[:, h * 64:(h + 1) * 64],
                                             start=(q8 == 0), stop=(q8 == 7))
                        nc.vector.tensor_scalar(
                            ztmp[0:64, half * 512:(half + 1) * 512], zp[:],
                            tyo[0:64, h:h + 1], None, AOT.is_ge)
                    if pr == 1:
                        nc.sync.dma_start(z_bf[hp][64:128, :], ztmp[:, :])

            # ---- phase 5: out = z @ proj_w.T + pb ----
            for nk in range(8):
                outs = work.tile([128, C], F32, name=f"outs{nk}", tag="outs")
                for half in range(2):
                    pp = mm.tile([128, 384], F32, name=f"pp{nk}_{half}", tag="mm")
                    for hp in range(6):
                        nc.tensor.matmul(pp[:], z_bf[hp][:, nk * 128:(nk + 1) * 128],
                                         p_hi[hp][:, half * 384:(half + 1) * 384],
                                         start=(hp == 0), stop=(hp == 5))
                    nc.vector.tensor_tensor(outs[:, half * 384:(half + 1) * 384],
                                            pp[:], pb[:, half * 384:(half + 1) * 384],
                                            AOT.add)
                nc.sync.dma_start(out_d[nk * 128:(nk + 1) * 128, :], outs[:])
    return nc


def prep_params(inputs):
    """Host-side folding of BN/Hoyer params + weight transposes/splits."""
    d = {k: np.asarray(v, np.float32) for k, v in inputs.items()}

    def fold(p, a):
        s = d[p + '_g'] / np.sqrt(d[p + '_v'] + EPS)
        thr = float(d[a + '_thr'])
        A = s / thr
        Bc = (d[p + '_b'] - d[p + '_m'] * s) / thr
        T2 = XS * d[a + '_run'] - Bc
        return A.astype(np.float32), T2.astype(np.float32)

    A_x, T2_x = fold('n', 'a')
    A_k, T2_k = fold('nk', 'ak')
    A_v, T2_v = fold('nv', 'av')
    A_o, T2_o = fold('no', 'ao')

    Wt = d['qkv_w'].T.copy()                       # [C, 3C]
    colscale = np.concatenate([np.ones(C, np.float32),
                               np.repeat(A_k, D), np.repeat(A_v, D)])
    Wt *= colscale[None, :]
    w_hi = Wt.astype(BF)
    w_lo = (Wt - w_hi.astype(np.float32)).astype(BF)

    p_hi = np.ascontiguousarray(d['proj_w'].T).astype(BF)

    def part6(vec):  # [768] -> [128, 6]; col ck = partition chunk ck
        return np.ascontiguousarray(vec.reshape(6, 128).T)

    return dict(
        w_hi=w_hi, w_lo=w_lo, p_hi=p_hi,
        txA=part6(np.repeat(A_x, D)), txT=part6(np.repeat(T2_x, D)),
        tkv=np.ascontiguousarray(np.broadcast_to(
            np.concatenate([np.repeat(T2_k, D), np.repeat(T2_v, D)]),
            (128, 2 * C))).astype(np.float32),
        qsc=part6(np.repeat(A_o, D)),
        tyO=np.ascontiguousarray(np.broadcast_to(T2_o, (128, H))).astype(np.float32),
        pb=np.ascontiguousarray(np.broadcast_to(d['proj_b'], (128, C))).astype(np.float32),
    )


def make_in_maps(inputs):
    shared = prep_params(inputs)
    x = np.asarray(inputs['x'], np.float32)
    return [dict(shared, xt=np.ascontiguousarray(x[c].T)) for c in range(NCORES)]


_CACHE = {}


def kernel(**inputs) -> np.ndarray:
    if 'exec' not in _CACHE:
        nc = build_nc()
        nc.compile()
        from bass_runner import make_executor
        run, run_dev, meta = make_executor(nc, NCORES)
        _CACHE['exec'] = (nc, run, run_dev, meta)
    nc, run, run_dev, meta = _CACHE['exec']
    in_maps = make_in_maps(inputs)
    results, _ = run(in_maps)
    return np.stack([results[c]['out'] for c in range(NCORES)]).astype(np.float32)
